# revision 4
# baseline (speedup 1.0000x reference)
"""CoralLoss (ordinal BCE-with-logits, mean reduction) on 8 Trainium2 cores.

Math: loss = mean over (B, K) of  max(x,0) - x*level + log1p(exp(-|x|))
where level[i,k] = (targets[i] > k).  Using softplus(x) = max(x,0) +
log1p(exp(-|x|)) = ln(1 + e^x), the total is

    sum(loss) = sum(softplus(x)) - sum(x * level)

Per-core kernel (data-parallel over B):
 - ScalarE computes sum(softplus(x)) as two chained activations per block
   (Exp, then Ln with bias=1 and fused accumulate).  e^x cannot overflow for
   randn-scale logits.
 - VectorE builds level masks per 128-row tile via one two-op tensor_scalar
   (iota < target) * -1.
 - PE contracts mask^T @ x into a PSUM (K,K) accumulator; its diagonal holds
   -sum(x * level) summed over all tiles.
 - A small finale reduces everything to one scalar per core; the host sums
   the 8 partials and divides by B*K.
"""

import numpy as np

import concourse.bacc as bacc
import concourse.bass_isa as bass_isa
import concourse.tile as tile
from concourse import mybir
from concourse.bass_utils import run_bass_kernel_spmd

B = 262144
K = 100
M = 8                      # cores
ROWS = B // M              # 32768 rows per core
P = 128                    # SBUF partitions
G = 16                     # 128-row tiles per block
NBLK = ROWS // (P * G)     # 16 blocks per core
CW = K + NBLK * G          # consts width: iota (K) + tcols (NBLK*G)

_NC_CACHE = {}


def _build_nc():
    nc = bacc.Bacc(None, target_bir_lowering=False)
    x_d = nc.dram_tensor("logits", [ROWS, K], mybir.dt.float32, kind="ExternalInput")
    c_d = nc.dram_tensor("consts", [P, CW], mybir.dt.float32, kind="ExternalInput")
    ident_d = nc.dram_tensor("ident", [K, K], mybir.dt.float32, kind="ExternalInput")
    out_d = nc.dram_tensor("partial", [1, 1], mybir.dt.float32, kind="ExternalOutput")

    # block b, partition p holds rows [b*P*G + p*G, b*P*G + (p+1)*G) contiguous
    xv = x_d.rearrange("(b p g) k -> b p (g k)", p=P, g=G)

    with tile.TileContext(nc) as tc:
        with (
            tc.tile_pool(name="xblk", bufs=3) as xpool,
            tc.tile_pool(name="singles", bufs=1) as spool,
            tc.tile_pool(name="mask", bufs=6) as mpool,
            tc.tile_pool(name="scr", bufs=2) as scrpool,
            tc.tile_pool(name="psum", bufs=1, space="PSUM") as ppool,
        ):
            consts_t = spool.tile([P, CW], mybir.dt.float32)
            nc.sync.dma_start(out=consts_t, in_=c_d[:, :])
            iota_t = consts_t[:, 0:K]
            tcols_t = consts_t[:, K:CW]
            ident_t = spool.tile([K, K], mybir.dt.float32)
            nc.sync.dma_start(out=ident_t, in_=ident_d[:, :])
            sp_cols = spool.tile([P, NBLK], mybir.dt.float32)

            psum_xl = ppool.tile([K, K], mybir.dt.float32)

            for b in range(NBLK):
                xblk = xpool.tile([P, G * K], mybir.dt.float32)
                nc.sync.dma_start(out=xblk, in_=xv[b])
                # softplus(x) = ln(1 + e^x); bf16 intermediate halves the Ln read
                u = scrpool.tile([P, G * K], mybir.dt.bfloat16)
                nc.scalar.activation(
                    out=u, in_=xblk, func=mybir.ActivationFunctionType.Exp
                )
                v = scrpool.tile([P, G * K], mybir.dt.bfloat16)
                nc.scalar.activation(
                    out=v,
                    in_=u,
                    func=mybir.ActivationFunctionType.Ln,
                    bias=1.0,
                    accum_out=sp_cols[:, b : b + 1],
                )
                for g in range(G):
                    j = b * G + g
                    mask = mpool.tile([P, K], mybir.dt.float32)
                    # mask = (iota < t) * -1.0  ->  {-1.0, 0.0}
                    nc.vector.tensor_scalar(
                        out=mask,
                        in0=iota_t,
                        scalar1=tcols_t[:, j : j + 1],
                        scalar2=-1.0,
                        op0=mybir.AluOpType.is_lt,
                        op1=mybir.AluOpType.mult,
                    )
                    nc.tensor.matmul(
                        out=psum_xl,
                        lhsT=mask,
                        rhs=xblk[:, g * K : (g + 1) * K],
                        start=(b == 0 and g == 0),
                        stop=(b == NBLK - 1 and g == G - 1),
                    )

            # finale: total = sum(sp_cols) + sum(diag(psum_xl))
            sp_row = spool.tile([P, 1], mybir.dt.float32)
            nc.vector.reduce_sum(out=sp_row, in_=sp_cols, axis=mybir.AxisListType.X)

            diag = spool.tile([P, K], mybir.dt.float32)
            nc.vector.memset(diag, 0.0)
            nc.vector.tensor_mul(diag[:K, :], psum_xl[:, :], ident_t[:, :])
            xl_row = spool.tile([P, 1], mybir.dt.float32)
            nc.vector.reduce_sum(out=xl_row, in_=diag, axis=mybir.AxisListType.X)

            tot = spool.tile([P, 1], mybir.dt.float32)
            nc.vector.tensor_add(tot, sp_row, xl_row)

            ones_t = spool.tile([P, 1], mybir.dt.float32)
            nc.vector.memset(ones_t, 1.0)
            psum_tot = ppool.tile([1, 1], mybir.dt.float32)
            nc.tensor.matmul(
                out=psum_tot, lhsT=tot, rhs=ones_t, start=True, stop=True
            )
            res = spool.tile([1, 1], mybir.dt.float32)
            nc.vector.tensor_copy(res, psum_tot)
            nc.sync.dma_start(out=out_d[:, :], in_=res)
    nc.finalize()
    return nc


def _run(logits, targets, trace=False, trace_kwargs=None):
    logits = np.ascontiguousarray(np.asarray(logits), dtype=np.float32)
    targets = np.asarray(targets)
    assert logits.shape == (B, K), logits.shape
    assert targets.shape == (B,), targets.shape

    if "nc" not in _NC_CACHE:
        _NC_CACHE["nc"] = _build_nc()
    nc = _NC_CACHE["nc"]

    iota = np.broadcast_to(np.arange(K, dtype=np.float32), (P, K))
    ident = np.eye(K, dtype=np.float32)
    t_f32 = targets.astype(np.float32)

    in_maps = []
    for c in range(M):
        xs = logits[c * ROWS : (c + 1) * ROWS]
        ts = t_f32[c * ROWS : (c + 1) * ROWS]
        tcols = ts.reshape(NBLK, P, G).transpose(1, 0, 2).reshape(P, NBLK * G)
        consts = np.concatenate([iota, tcols], axis=1)
        in_maps.append({"logits": xs, "consts": consts, "ident": ident})

    res = run_bass_kernel_spmd(
        nc, in_maps, core_ids=list(range(M)), trace=trace, **(trace_kwargs or {})
    )
    total = sum(float(res.results[c]["partial"][0, 0]) for c in range(M))
    out = np.array(total / (B * K), dtype=np.float32)
    return out, res


def kernel(logits, targets):
    out, _ = _run(logits, targets)
    return out
